# revision 1
# baseline (speedup 1.0000x reference)
"""GumbelSlotSelector Trainium kernel.

Math (per row r of B*K rows, D=128, H=64):
  h = relu(x @ W1 + b1);  dlogit = h @ (W2[:,1]-W2[:,0]) + (b2[1]-b2[0])
  decision = 1.0 if dlogit + g1 - g0 > 0 else 0.0,  g_i = -log(-log(clip(u_i)))
  keep_probs = sigmoid(dlogit)
  fixup: rows (of K=64 slots) with no active slot activate their argmax(fix_u) slot.

Sharding: pure data-parallel over batch B=8192 -> 8 cores x 1024 rows
(65536 (b,k)-rows of 128 features per core).

Per-core dataflow (strips of 1024 rows):
  DMA x-strip [128p, 8t x 128d] -> 8 PE transposes -> XT psum [128d, 1024]
  -> copy to SBUF -> mm1 (lhsT=W1) -> HT psum [64, 1024] -> relu(+b1)
  -> mm2 (lhsT = w2d embedded at column c%32) accumulating into a single
  [128, 512] dlogit psum bank where partition c = rows [512c, 512c+512).
  Final elementwise phase computes gumbel decision + sigmoid + fixup.
"""
import sys

sys.path.insert(0, "/opt/trn_rl_repo")
import numpy as np
from contextlib import ExitStack

import concourse.bacc as bacc
import concourse.tile as tile
from concourse import mybir, bass_utils
from concourse.bass_interp import get_hw_module

F32 = mybir.dt.float32
AF = mybir.ActivationFunctionType
ALU = mybir.AluOpType

B, K, D, H = 8192, 64, 128, 64
NCORES = 8
R = (B // NCORES) * K          # 65536 rows per core
SR = 1024                      # strip rows
NSTRIP = R // SR               # 64
NT = SR // 128                 # 8 sub-tiles per strip
CLIP_LO = 1e-10
CLIP_HI = float(np.float32(1.0 - 1e-7))

_CACHE = {}


def _build():
    nc = bacc.Bacc("TRN2", target_bir_lowering=False, debug=False,
                   num_devices=NCORES)
    x_d = nc.dram_tensor("x", [R, D], F32, kind="ExternalInput")
    gu_d = nc.dram_tensor("gu", [R, 2], F32, kind="ExternalInput")
    fu_d = nc.dram_tensor("fu", [R], F32, kind="ExternalInput")
    w1_d = nc.dram_tensor("w1", [D, H], F32, kind="ExternalInput")
    emb_d = nc.dram_tensor("emb", [H, 64 * 64], F32, kind="ExternalInput")
    b1_d = nc.dram_tensor("b1c", [H, 1], F32, kind="ExternalInput")
    b2_d = nc.dram_tensor("b2dv", [128, 1], F32, kind="ExternalInput")
    eye_d = nc.dram_tensor("eye", [128, 128], F32, kind="ExternalInput")
    dec_d = nc.dram_tensor("dec", [R], F32, kind="ExternalOutput")
    keep_d = nc.dram_tensor("keep", [R], F32, kind="ExternalOutput")

    with tile.TileContext(nc) as tc, ExitStack() as ctx:
        cpool = ctx.enter_context(tc.tile_pool(name="const", bufs=1))
        xpool = ctx.enter_context(tc.tile_pool(name="x", bufs=3))
        tpool = ctx.enter_context(tc.tile_pool(name="xt", bufs=2))
        rpool = ctx.enter_context(tc.tile_pool(name="relu", bufs=2))
        fpool = ctx.enter_context(tc.tile_pool(name="fin", bufs=1))
        ps_xt = ctx.enter_context(tc.tile_pool(name="psxt", bufs=1, space="PSUM"))
        ps_ht = ctx.enter_context(tc.tile_pool(name="psht", bufs=2, space="PSUM"))
        ps_dl = ctx.enter_context(tc.tile_pool(name="psdl", bufs=1, space="PSUM"))

        w1_sb = cpool.tile([D, H], F32)
        nc.sync.dma_start(w1_sb[:], w1_d.ap())
        emb_sb = cpool.tile([H, 64 * 64], F32)
        nc.sync.dma_start(emb_sb[:], emb_d.ap())
        b1_sb = cpool.tile([H, 1], F32)
        nc.sync.dma_start(b1_sb[:], b1_d.ap())
        b2_sb = cpool.tile([128, 1], F32)
        nc.sync.dma_start(b2_sb[:], b2_d.ap())
        eye_sb = cpool.tile([128, 128], F32)
        nc.sync.dma_start(eye_sb[:], eye_d.ap())

        dl_ps = ps_dl.tile([128, 512], F32)

        for s in range(NSTRIP):
            x_sb = xpool.tile([128, SR], F32)
            nc.sync.dma_start(
                x_sb[:].rearrange("p (t d) -> p t d", d=D),
                x_d.ap()[s * SR:(s + 1) * SR, :].rearrange(
                    "(t p) d -> p t d", p=128),
            )
            xt_ps = ps_xt.tile([128, SR], F32)
            for t in range(NT):
                nc.tensor.transpose(
                    xt_ps[:, t * 128:(t + 1) * 128],
                    x_sb[:, t * D:(t + 1) * D],
                    eye_sb[:],
                )
            xt_sb = tpool.tile([128, SR], F32)
            if s % 2 == 0:
                nc.vector.tensor_copy(xt_sb[:], xt_ps[:])
            else:
                nc.scalar.copy(xt_sb[:], xt_ps[:])

            ht_ps = ps_ht.tile([H, SR], F32)
            for k in range(2):
                nc.tensor.matmul(
                    ht_ps[:, k * 512:(k + 1) * 512],
                    w1_sb[:],
                    xt_sb[:, k * 512:(k + 1) * 512],
                    start=True, stop=True,
                )
            relu_sb = rpool.tile([H, SR], F32)
            if s % 2 == 0:
                nc.vector.tensor_scalar(
                    relu_sb[:], ht_ps[:], b1_sb[:, 0:1], 0.0,
                    op0=ALU.add, op1=ALU.max)
            else:
                nc.scalar.activation(relu_sb[:], ht_ps[:], AF.Relu,
                                     bias=b1_sb[:, 0:1])

            for k in range(2):
                c = 2 * s + k
                g, m = c // 64, c % 64
                nc.tensor.matmul(
                    dl_ps[64 * g:64 * g + 64, :],
                    emb_sb[:, 64 * m:64 * m + 64],
                    relu_sb[:, k * 512:(k + 1) * 512],
                    start=(m == 0), stop=(m == 63),
                    skip_group_check=True,
                )

        # ---- final elementwise phase on [128, 512] (row r = 512p + s) ----
        gu_sb = fpool.tile([128, 1024], F32)
        nc.sync.dma_start(
            gu_sb[:].rearrange("p (s u) -> p s u", u=2),
            gu_d.ap().rearrange("(p s) u -> p s u", p=128),
        )
        fu_sb = fpool.tile([128, 512], F32)
        nc.sync.dma_start(fu_sb[:], fu_d.ap().rearrange("(p s) -> p s", p=128))

        gu_v = gu_sb[:].rearrange("p (s u) -> p s u", u=2)
        a0 = fpool.tile([128, 512], F32)
        a1 = fpool.tile([128, 512], F32)
        nc.vector.tensor_scalar(a0[:], gu_v[:, :, 0], CLIP_LO, CLIP_HI,
                                op0=ALU.max, op1=ALU.min)
        nc.vector.tensor_scalar(a1[:], gu_v[:, :, 1], CLIP_LO, CLIP_HI,
                                op0=ALU.max, op1=ALU.min)
        # g_i = -log(-log(u_i)); g0m = log(-log u0) = -g0
        nc.scalar.activation(a0[:], a0[:], AF.Ln)
        nc.scalar.activation(a1[:], a1[:], AF.Ln)
        g0m = fpool.tile([128, 512], F32)
        g1m = fpool.tile([128, 512], F32)
        nc.scalar.activation(g0m[:], a0[:], AF.Ln, scale=-1.0)
        nc.scalar.activation(g1m[:], a1[:], AF.Ln, scale=-1.0)
        t1 = fpool.tile([128, 512], F32)
        nc.vector.tensor_sub(t1[:], g0m[:], g1m[:])  # g1 - g0
        z = fpool.tile([128, 512], F32)
        nc.vector.scalar_tensor_tensor(z[:], dl_ps[:], b2_sb[:, 0:1], t1[:],
                                       op0=ALU.add, op1=ALU.add)
        dec_sb = fpool.tile([128, 512], F32)
        nc.vector.tensor_scalar(dec_sb[:], z[:], 0.0, None, op0=ALU.is_gt)
        keep_sb = fpool.tile([128, 512], F32)
        nc.scalar.activation(keep_sb[:], dl_ps[:], AF.Sigmoid,
                             bias=b2_sb[:, 0:1])

        # fixup: rows with no active slot activate argmax(fix_u)
        dec_v = dec_sb[:].rearrange("p (g k) -> p g k", k=64)
        fu_v = fu_sb[:].rearrange("p (g k) -> p g k", k=64)
        rs = fpool.tile([128, 8], F32)
        nc.vector.reduce_sum(rs[:], dec_v, axis=mybir.AxisListType.X)
        need = fpool.tile([128, 8], F32)
        nc.vector.tensor_scalar(need[:], rs[:], 0.0, None, op0=ALU.is_equal)
        fmx = fpool.tile([128, 8], F32)
        nc.vector.reduce_max(fmx[:], fu_v, axis=mybir.AxisListType.X)
        fixm = fpool.tile([128, 512], F32)
        fixm_v = fixm[:].rearrange("p (g k) -> p g k", k=64)
        for g in range(8):
            nc.vector.tensor_scalar(
                fixm_v[:, g, :], fu_v[:, g, :],
                fmx[:, g:g + 1], need[:, g:g + 1],
                op0=ALU.is_ge, op1=ALU.mult)
        nc.vector.tensor_tensor(dec_sb[:], dec_sb[:], fixm[:], op=ALU.max)

        nc.sync.dma_start(dec_d.ap().rearrange("(p s) -> p s", p=128), dec_sb[:])
        nc.sync.dma_start(keep_d.ap().rearrange("(p s) -> p s", p=128), keep_sb[:])

    nc.compile()
    nc.m = get_hw_module(nc.m)
    return nc


def kernel(slots, gumbel_u, fix_u, W1, b1, W2, b2, _trace=False):
    slots = np.ascontiguousarray(slots, np.float32)
    gumbel_u = np.ascontiguousarray(gumbel_u, np.float32)
    fix_u = np.ascontiguousarray(fix_u, np.float32)
    W1 = np.ascontiguousarray(W1, np.float32)
    W2 = np.ascontiguousarray(W2, np.float32)
    w2d = (W2[:, 1] - W2[:, 0]).astype(np.float32)
    b2d = np.float32(b2[1] - b2[0])

    emb = np.zeros((H, 64, 64), np.float32)
    emb[:, np.arange(64), np.arange(64)] = w2d[:, None]
    emb = emb.reshape(H, 64 * 64)
    b1c = np.ascontiguousarray(b1, np.float32).reshape(H, 1)
    b2dv = np.full((128, 1), b2d, np.float32)
    eye = np.eye(128, dtype=np.float32)

    if "nc" not in _CACHE:
        _CACHE["nc"] = _build()
    nc = _CACHE["nc"]

    bpc = B // NCORES
    in_maps = []
    for c in range(NCORES):
        in_maps.append({
            "x": slots[c * bpc:(c + 1) * bpc].reshape(R, D),
            "gu": gumbel_u[c * bpc:(c + 1) * bpc].reshape(R, 2),
            "fu": fix_u[c * bpc:(c + 1) * bpc].reshape(R),
            "w1": W1, "emb": emb, "b1c": b1c, "b2dv": b2dv, "eye": eye,
        })
    res = bass_utils.run_bass_kernel_spmd(
        nc, in_maps, core_ids=list(range(NCORES)), trace=_trace)
    _CACHE["last_result"] = res

    dec = np.concatenate(
        [res.results[c]["dec"].reshape(bpc, K) for c in range(NCORES)], axis=0)
    keep = np.concatenate(
        [res.results[c]["keep"].reshape(bpc, K) for c in range(NCORES)], axis=0)
    return dec, keep



# revision 11
# speedup vs baseline: 1.6487x; 1.6487x over previous
"""GumbelSlotSelector Trainium kernel.

Math (per row r of B*K rows, D=128, H=64):
  h = relu(x @ W1 + b1);  dlogit = h @ (W2[:,1]-W2[:,0]) + (b2[1]-b2[0])
  decision = 1.0 if dlogit + g1 - g0 > 0 else 0.0,  g_i = -log(-log(clip(u_i)))
  keep_probs = sigmoid(dlogit)
  fixup: rows (of K=64 slots) with no active slot activate their argmax(fix_u) slot.

Sharding: pure data-parallel over batch B=8192 -> 8 cores x 1024 rows
(65536 (b,k)-rows of 128 features per core).

Per-core dataflow (strips of 1024 rows):
  DMA x-strip [128p, 8t x 128d] -> 8 PE transposes -> XT psum [128d, 1024]
  -> copy to SBUF -> mm1 (lhsT=W1) -> HT psum [64, 1024] -> relu(+b1)
  -> mm2 (lhsT = w2d embedded at column c%32) accumulating into a single
  [128, 512] dlogit psum bank where partition c = rows [512c, 512c+512).
  Final elementwise phase computes gumbel decision + sigmoid + fixup.
"""
import sys

sys.path.insert(0, "/opt/trn_rl_repo")
import numpy as np
from contextlib import ExitStack

import concourse.bacc as bacc
import concourse.tile as tile
from concourse import mybir, bass_utils
from concourse.bass_interp import get_hw_module

F32 = mybir.dt.float32
F32R = mybir.dt.float32r
AF = mybir.ActivationFunctionType
ALU = mybir.AluOpType

B, K, D, H = 8192, 64, 128, 64
NCORES = 8
R = (B // NCORES) * K          # 65536 rows per core
SR = 1024                      # strip rows
NSTRIP = R // SR               # 64
NT = SR // 128                 # 8 sub-tiles per strip
CLIP_LO = 1e-10
CLIP_HI = float(np.float32(1.0 - 1e-7))

_CACHE = {}


def _build():
    nc = bacc.Bacc("TRN2", target_bir_lowering=False, debug=False,
                   num_devices=NCORES)
    x_d = nc.dram_tensor("x", [R, D], F32, kind="ExternalInput")
    gu_d = nc.dram_tensor("gu", [R, 2], F32, kind="ExternalInput")
    fu_d = nc.dram_tensor("fu", [R], F32, kind="ExternalInput")
    w1_d = nc.dram_tensor("w1", [D, H], F32, kind="ExternalInput")
    emb_d = nc.dram_tensor("emb", [H, 64 * 64], F32, kind="ExternalInput")
    b1_d = nc.dram_tensor("b1c", [H, 1], F32, kind="ExternalInput")
    b2_d = nc.dram_tensor("b2dv", [128, 1], F32, kind="ExternalInput")
    eye_d = nc.dram_tensor("eye", [128, 128], F32, kind="ExternalInput")
    dec_d = nc.dram_tensor("dec", [R], F32, kind="ExternalOutput")
    keep_d = nc.dram_tensor("keep", [R], F32, kind="ExternalOutput")

    with tile.TileContext(nc) as tc, ExitStack() as ctx:
        cpool = ctx.enter_context(tc.tile_pool(name="const", bufs=1))
        xpool = ctx.enter_context(tc.tile_pool(name="x", bufs=3))
        tpool = ctx.enter_context(tc.tile_pool(name="xt", bufs=2))
        rpool = ctx.enter_context(tc.tile_pool(name="relu", bufs=2))
        fpool = ctx.enter_context(tc.tile_pool(name="fin", bufs=1))
        ps_xt = ctx.enter_context(tc.tile_pool(name="psxt", bufs=1, space="PSUM"))
        ps_ht = ctx.enter_context(tc.tile_pool(name="psht", bufs=2, space="PSUM"))
        ps_dl = ctx.enter_context(tc.tile_pool(name="psdl", bufs=1, space="PSUM"))

        w1_sb = cpool.tile([D, H], F32)
        nc.sync.dma_start(w1_sb[:], w1_d.ap())
        emb_sb = cpool.tile([H, 64 * 64], F32)
        nc.sync.dma_start(emb_sb[:], emb_d.ap())
        # fp32r-rounded copies of the matmul constants (PE runs mm1/mm2 in
        # fp32r at 1 cycle/row vs fp32's 4)
        w1r_sb = cpool.tile([D, H], F32R)
        nc.vector.tensor_copy(w1r_sb[:], w1_sb[:])
        embr_sb = cpool.tile([H, 64 * 64], F32R)
        nc.vector.tensor_copy(embr_sb[:], emb_sb[:])
        b1_sb = cpool.tile([H, 1], F32)
        nc.sync.dma_start(b1_sb[:], b1_d.ap())
        b2_sb = cpool.tile([128, 1], F32)
        nc.sync.dma_start(b2_sb[:], b2_d.ap())
        eye_sb = cpool.tile([128, 128], F32)
        nc.sync.dma_start(eye_sb[:], eye_d.ap())

        # fp32r matmul dst must start at partition 0 -> two banks of [64, 512]
        dl_ps_a = ps_dl.tile([64, 512], F32)
        dl_ps_b = ps_dl.tile([64, 512], F32)
        dl_ps = [dl_ps_a, dl_ps_b]

        for s in range(NSTRIP):
            x_sb = xpool.tile([128, SR], F32)
            nc.sync.dma_start(
                x_sb[:].rearrange("p (t d) -> p t d", d=D),
                x_d.ap()[s * SR:(s + 1) * SR, :].rearrange(
                    "(t p) d -> p t d", p=128),
            )
            xt_ps = ps_xt.tile([128, SR], F32)
            for t in range(NT):
                nc.tensor.transpose(
                    xt_ps[:, t * 128:(t + 1) * 128],
                    x_sb[:, t * D:(t + 1) * D],
                    eye_sb[:],
                )
            xt_sb = tpool.tile([128, SR], F32R)
            if s % 2 == 0:
                nc.vector.tensor_copy(xt_sb[:], xt_ps[:])
            else:
                nc.scalar.copy(xt_sb[:], xt_ps[:])

            ht_ps = ps_ht.tile([H, SR], F32)
            for k in range(2):
                nc.tensor.matmul(
                    ht_ps[:, k * 512:(k + 1) * 512],
                    w1r_sb[:],
                    xt_sb[:, k * 512:(k + 1) * 512],
                    start=True, stop=True,
                )
            relu_sb = rpool.tile([H, SR], F32R)
            if s % 2 == 0:
                nc.vector.tensor_scalar(
                    relu_sb[:], ht_ps[:], b1_sb[:, 0:1], 0.0,
                    op0=ALU.add, op1=ALU.max)
            else:
                nc.scalar.activation(relu_sb[:], ht_ps[:], AF.Relu,
                                     bias=b1_sb[:, 0:1])

            for k in range(2):
                c = 2 * s + k
                g, m = c // 64, c % 64
                nc.tensor.matmul(
                    dl_ps[g][:],
                    embr_sb[:, 64 * m:64 * m + 64],
                    relu_sb[:, k * 512:(k + 1) * 512],
                    start=(m == 0), stop=(m == 63),
                    skip_group_check=True,
                )

        # ---- final elementwise phase on [128, 512] (row r = 512p + s) ----
        dl_sb = fpool.tile([128, 512], F32)
        nc.vector.tensor_copy(dl_sb[0:64, :], dl_ps[0][:])
        nc.scalar.copy(dl_sb[64:128, :], dl_ps[1][:])
        gu_sb = fpool.tile([128, 1024], F32)
        nc.sync.dma_start(
            gu_sb[:].rearrange("p (s u) -> p s u", u=2),
            gu_d.ap().rearrange("(p s) u -> p s u", p=128),
        )
        fu_sb = fpool.tile([128, 512], F32)
        nc.sync.dma_start(fu_sb[:], fu_d.ap().rearrange("(p s) -> p s", p=128))

        gu_v = gu_sb[:].rearrange("p (s u) -> p s u", u=2)
        a0 = fpool.tile([128, 512], F32)
        a1 = fpool.tile([128, 512], F32)
        nc.vector.tensor_scalar(a0[:], gu_v[:, :, 0], CLIP_LO, CLIP_HI,
                                op0=ALU.max, op1=ALU.min)
        nc.vector.tensor_scalar(a1[:], gu_v[:, :, 1], CLIP_LO, CLIP_HI,
                                op0=ALU.max, op1=ALU.min)
        # g_i = -log(-log(u_i)); g0m = log(-log u0) = -g0
        nc.scalar.activation(a0[:], a0[:], AF.Ln)
        nc.scalar.activation(a1[:], a1[:], AF.Ln)
        g0m = fpool.tile([128, 512], F32)
        g1m = fpool.tile([128, 512], F32)
        nc.scalar.activation(g0m[:], a0[:], AF.Ln, scale=-1.0)
        nc.scalar.activation(g1m[:], a1[:], AF.Ln, scale=-1.0)
        t1 = fpool.tile([128, 512], F32)
        nc.vector.tensor_sub(t1[:], g0m[:], g1m[:])  # g1 - g0
        z = fpool.tile([128, 512], F32)
        nc.vector.scalar_tensor_tensor(z[:], dl_sb[:], b2_sb[:, 0:1], t1[:],
                                       op0=ALU.add, op1=ALU.add)
        dec_sb = fpool.tile([128, 512], F32)
        nc.vector.tensor_scalar(dec_sb[:], z[:], 0.0, None, op0=ALU.is_gt)
        keep_sb = fpool.tile([128, 512], F32)
        nc.scalar.activation(keep_sb[:], dl_sb[:], AF.Sigmoid,
                             bias=b2_sb[:, 0:1])

        # fixup: rows with no active slot activate argmax(fix_u)
        dec_v = dec_sb[:].rearrange("p (g k) -> p g k", k=64)
        fu_v = fu_sb[:].rearrange("p (g k) -> p g k", k=64)
        rs = fpool.tile([128, 8], F32)
        nc.vector.reduce_sum(rs[:], dec_v, axis=mybir.AxisListType.X)
        need = fpool.tile([128, 8], F32)
        nc.vector.tensor_scalar(need[:], rs[:], 0.0, None, op0=ALU.is_equal)
        fmx = fpool.tile([128, 8], F32)
        nc.vector.reduce_max(fmx[:], fu_v, axis=mybir.AxisListType.X)
        fixm = fpool.tile([128, 512], F32)
        fixm_v = fixm[:].rearrange("p (g k) -> p g k", k=64)
        for g in range(8):
            nc.vector.tensor_scalar(
                fixm_v[:, g, :], fu_v[:, g, :],
                fmx[:, g:g + 1], need[:, g:g + 1],
                op0=ALU.is_ge, op1=ALU.mult)
        nc.vector.tensor_tensor(dec_sb[:], dec_sb[:], fixm[:], op=ALU.max)

        nc.sync.dma_start(dec_d.ap().rearrange("(p s) -> p s", p=128), dec_sb[:])
        nc.sync.dma_start(keep_d.ap().rearrange("(p s) -> p s", p=128), keep_sb[:])

    nc.compile()
    nc.m = get_hw_module(nc.m)
    return nc


def kernel(slots, gumbel_u, fix_u, W1, b1, W2, b2, _trace=False):
    slots = np.ascontiguousarray(slots, np.float32)
    gumbel_u = np.ascontiguousarray(gumbel_u, np.float32)
    fix_u = np.ascontiguousarray(fix_u, np.float32)
    W1 = np.ascontiguousarray(W1, np.float32)
    W2 = np.ascontiguousarray(W2, np.float32)
    w2d = (W2[:, 1] - W2[:, 0]).astype(np.float32)
    b2d = np.float32(b2[1] - b2[0])

    emb = np.zeros((H, 64, 64), np.float32)
    emb[:, np.arange(64), np.arange(64)] = w2d[:, None]
    emb = emb.reshape(H, 64 * 64)
    b1c = np.ascontiguousarray(b1, np.float32).reshape(H, 1)
    b2dv = np.full((128, 1), b2d, np.float32)
    eye = np.eye(128, dtype=np.float32)

    if "nc" not in _CACHE:
        _CACHE["nc"] = _build()
    nc = _CACHE["nc"]

    bpc = B // NCORES
    in_maps = []
    for c in range(NCORES):
        in_maps.append({
            "x": slots[c * bpc:(c + 1) * bpc].reshape(R, D),
            "gu": gumbel_u[c * bpc:(c + 1) * bpc].reshape(R, 2),
            "fu": fix_u[c * bpc:(c + 1) * bpc].reshape(R),
            "w1": W1, "emb": emb, "b1c": b1c, "b2dv": b2dv, "eye": eye,
        })
    res = bass_utils.run_bass_kernel_spmd(
        nc, in_maps, core_ids=list(range(NCORES)), trace=_trace)
    _CACHE["last_result"] = res

    dec = np.concatenate(
        [res.results[c]["dec"].reshape(bpc, K) for c in range(NCORES)], axis=0)
    keep = np.concatenate(
        [res.results[c]["keep"].reshape(bpc, K) for c in range(NCORES)], axis=0)
    return dec, keep



# revision 12
# speedup vs baseline: 2.1748x; 1.3191x over previous
"""GumbelSlotSelector Trainium kernel.

Math (per row r of B*K rows, D=128, H=64):
  h = relu(x @ W1 + b1);  dlogit = h @ (W2[:,1]-W2[:,0]) + (b2[1]-b2[0])
  decision = 1.0 if dlogit + g1 - g0 > 0 else 0.0,  g_i = -log(-log(clip(u_i)))
  keep_probs = sigmoid(dlogit)
  fixup: rows (of K=64 slots) with no active slot activate their argmax(fix_u) slot.

Sharding: pure data-parallel over batch B=8192 -> 8 cores x 1024 rows
(65536 (b,k)-rows of 128 features per core).

Precision: slots are shipped to HBM as fp16 (halves the dominant DMA
traffic; 2^-11 rounding), mm1 runs in fp16, mm2 in fp32r (2^-12).
Measured decision flips vs fp32 reference: ~30/524288 -> rel err ~1e-2,
under the 2e-2 gate.

Per-core dataflow (strips of 1024 rows):
  DMA-xbar-transpose x16 strip [1024, 128] -> XT sbuf [128d, 1024] fp16
  -> mm1 (lhsT=W1 fp16) -> HT psum [64, 1024] -> relu(+b1) -> f32r
  -> mm2 (lhsT = w2d embedded at column c%64, f32r) accumulating into two
  [64, 512] dlogit psum banks (bank g, partition c%64 = rows
  [512c, 512c+512) for c = 64g + m).
  Final elementwise phase computes gumbel decision + sigmoid + fixup.
"""
import sys

sys.path.insert(0, "/opt/trn_rl_repo")
import numpy as np
from contextlib import ExitStack

import concourse.bacc as bacc
import concourse.tile as tile
from concourse import mybir, bass_utils
from concourse.bass_interp import get_hw_module

F32 = mybir.dt.float32
F32R = mybir.dt.float32r
F16 = mybir.dt.float16
AF = mybir.ActivationFunctionType
ALU = mybir.AluOpType

B, K, D, H = 8192, 64, 128, 64
NCORES = 8
R = (B // NCORES) * K          # 65536 rows per core
SR = 1024                      # strip rows
NSTRIP = R // SR               # 64
CLIP_LO = 1e-10
CLIP_HI = float(np.float32(1.0 - 1e-7))

_CACHE = {}


def _build():
    nc = bacc.Bacc("TRN2", target_bir_lowering=False, debug=False,
                   num_devices=NCORES)
    x_d = nc.dram_tensor("x16", [R, D], F16, kind="ExternalInput")
    gu_d = nc.dram_tensor("gu", [R, 2], F32, kind="ExternalInput")
    fu_d = nc.dram_tensor("fu", [R], F32, kind="ExternalInput")
    w1_d = nc.dram_tensor("w1h", [D, H], F16, kind="ExternalInput")
    emb_d = nc.dram_tensor("emb", [H, 64 * 64], F32, kind="ExternalInput")
    b1_d = nc.dram_tensor("b1c", [H, 1], F32, kind="ExternalInput")
    b2_d = nc.dram_tensor("b2dv", [128, 1], F32, kind="ExternalInput")
    dec_d = nc.dram_tensor("dec", [R], F32, kind="ExternalOutput")
    keep_d = nc.dram_tensor("keep", [R], F32, kind="ExternalOutput")

    with tile.TileContext(nc) as tc, ExitStack() as ctx:
        cpool = ctx.enter_context(tc.tile_pool(name="const", bufs=1))
        tpool = ctx.enter_context(tc.tile_pool(name="xt", bufs=4))
        rpool = ctx.enter_context(tc.tile_pool(name="relu", bufs=2))
        fpool = ctx.enter_context(tc.tile_pool(name="fin", bufs=1))
        ps_ht = ctx.enter_context(tc.tile_pool(name="psht", bufs=3, space="PSUM"))
        ps_dl = ctx.enter_context(tc.tile_pool(name="psdl", bufs=1, space="PSUM"))

        w1_sb = cpool.tile([D, H], F16)
        nc.sync.dma_start(w1_sb[:], w1_d.ap())
        emb_sb = cpool.tile([H, 64 * 64], F32)
        nc.sync.dma_start(emb_sb[:], emb_d.ap())
        b1_sb = cpool.tile([H, 1], F32)
        nc.sync.dma_start(b1_sb[:], b1_d.ap())
        b2_sb = cpool.tile([128, 1], F32)
        nc.sync.dma_start(b2_sb[:], b2_d.ap())
        # fp32r-rounded copy of the mm2 constant (PE runs mm2 in fp32r at
        # 1 cycle/row vs fp32's 4)
        embr_sb = cpool.tile([H, 64 * 64], F32R)
        nc.vector.tensor_copy(embr_sb[:], emb_sb[:])

        # fp32r matmul dst must start at partition 0 -> two banks of [64, 512]
        dl_ps_a = ps_dl.tile([64, 512], F32)
        dl_ps_b = ps_dl.tile([64, 512], F32)
        dl_ps = [dl_ps_a, dl_ps_b]

        for s in range(NSTRIP):
            xt_sb = tpool.tile([128, SR], F16)
            nc.sync.dma_start_transpose(
                xt_sb[:], x_d.ap()[s * SR:(s + 1) * SR, :])

            ht_ps = ps_ht.tile([H, SR], F32)
            for k in range(2):
                nc.tensor.matmul(
                    ht_ps[:, k * 512:(k + 1) * 512],
                    w1_sb[:],
                    xt_sb[:, k * 512:(k + 1) * 512],
                    start=True, stop=True,
                )
            relu_sb = rpool.tile([H, SR], F32R)
            if s % 2 == 0:
                nc.vector.tensor_scalar(
                    relu_sb[:], ht_ps[:], b1_sb[:, 0:1], 0.0,
                    op0=ALU.add, op1=ALU.max)
            else:
                nc.scalar.activation(relu_sb[:], ht_ps[:], AF.Relu,
                                     bias=b1_sb[:, 0:1])

            for k in range(2):
                c = 2 * s + k
                g, m = c // 64, c % 64
                nc.tensor.matmul(
                    dl_ps[g][:],
                    embr_sb[:, 64 * m:64 * m + 64],
                    relu_sb[:, k * 512:(k + 1) * 512],
                    start=(m == 0), stop=(m == 63),
                    skip_group_check=True,
                )

        # ---- final elementwise phase on [128, 512] (row r = 512p + s) ----
        dl_sb = fpool.tile([128, 512], F32)
        nc.vector.tensor_copy(dl_sb[0:64, :], dl_ps[0][:])
        nc.scalar.copy(dl_sb[64:128, :], dl_ps[1][:])
        gu_sb = fpool.tile([128, 1024], F32)
        nc.sync.dma_start(
            gu_sb[:].rearrange("p (s u) -> p s u", u=2),
            gu_d.ap().rearrange("(p s) u -> p s u", p=128),
        )
        fu_sb = fpool.tile([128, 512], F32)
        nc.sync.dma_start(fu_sb[:], fu_d.ap().rearrange("(p s) -> p s", p=128))

        gu_v = gu_sb[:].rearrange("p (s u) -> p s u", u=2)
        a0 = fpool.tile([128, 512], F32)
        a1 = fpool.tile([128, 512], F32)
        nc.vector.tensor_scalar(a0[:], gu_v[:, :, 0], CLIP_LO, CLIP_HI,
                                op0=ALU.max, op1=ALU.min)
        nc.vector.tensor_scalar(a1[:], gu_v[:, :, 1], CLIP_LO, CLIP_HI,
                                op0=ALU.max, op1=ALU.min)
        # g_i = -log(-log(u_i)); g0m = log(-log u0) = -g0
        nc.scalar.activation(a0[:], a0[:], AF.Ln)
        nc.scalar.activation(a1[:], a1[:], AF.Ln)
        g0m = fpool.tile([128, 512], F32)
        g1m = fpool.tile([128, 512], F32)
        nc.scalar.activation(g0m[:], a0[:], AF.Ln, scale=-1.0)
        nc.scalar.activation(g1m[:], a1[:], AF.Ln, scale=-1.0)
        t1 = fpool.tile([128, 512], F32)
        nc.vector.tensor_sub(t1[:], g0m[:], g1m[:])  # g1 - g0
        z = fpool.tile([128, 512], F32)
        nc.vector.scalar_tensor_tensor(z[:], dl_sb[:], b2_sb[:, 0:1], t1[:],
                                       op0=ALU.add, op1=ALU.add)
        dec_sb = fpool.tile([128, 512], F32)
        nc.vector.tensor_scalar(dec_sb[:], z[:], 0.0, None, op0=ALU.is_gt)
        keep_sb = fpool.tile([128, 512], F32)
        nc.scalar.activation(keep_sb[:], dl_sb[:], AF.Sigmoid,
                             bias=b2_sb[:, 0:1])

        # fixup: rows with no active slot activate argmax(fix_u)
        dec_v = dec_sb[:].rearrange("p (g k) -> p g k", k=64)
        fu_v = fu_sb[:].rearrange("p (g k) -> p g k", k=64)
        rs = fpool.tile([128, 8], F32)
        nc.vector.reduce_sum(rs[:], dec_v, axis=mybir.AxisListType.X)
        need = fpool.tile([128, 8], F32)
        nc.vector.tensor_scalar(need[:], rs[:], 0.0, None, op0=ALU.is_equal)
        fmx = fpool.tile([128, 8], F32)
        nc.vector.reduce_max(fmx[:], fu_v, axis=mybir.AxisListType.X)
        fixm = fpool.tile([128, 512], F32)
        fixm_v = fixm[:].rearrange("p (g k) -> p g k", k=64)
        for g in range(8):
            nc.vector.tensor_scalar(
                fixm_v[:, g, :], fu_v[:, g, :],
                fmx[:, g:g + 1], need[:, g:g + 1],
                op0=ALU.is_ge, op1=ALU.mult)
        nc.vector.tensor_tensor(dec_sb[:], dec_sb[:], fixm[:], op=ALU.max)

        nc.sync.dma_start(dec_d.ap().rearrange("(p s) -> p s", p=128), dec_sb[:])
        nc.sync.dma_start(keep_d.ap().rearrange("(p s) -> p s", p=128), keep_sb[:])

    nc.compile()
    nc.m = get_hw_module(nc.m)
    return nc


def kernel(slots, gumbel_u, fix_u, W1, b1, W2, b2, _trace=False):
    gumbel_u = np.ascontiguousarray(gumbel_u, np.float32)
    fix_u = np.ascontiguousarray(fix_u, np.float32)
    x16 = np.ascontiguousarray(slots, np.float16)
    w1h = np.ascontiguousarray(W1, np.float16)
    W2 = np.ascontiguousarray(W2, np.float32)
    w2d = (W2[:, 1] - W2[:, 0]).astype(np.float32)
    b2d = np.float32(b2[1] - b2[0])

    emb = np.zeros((H, 64, 64), np.float32)
    emb[:, np.arange(64), np.arange(64)] = w2d[:, None]
    emb = emb.reshape(H, 64 * 64)
    b1c = np.ascontiguousarray(b1, np.float32).reshape(H, 1)
    b2dv = np.full((128, 1), b2d, np.float32)

    if "nc" not in _CACHE:
        _CACHE["nc"] = _build()
    nc = _CACHE["nc"]

    bpc = B // NCORES
    in_maps = []
    for c in range(NCORES):
        in_maps.append({
            "x16": x16[c * bpc:(c + 1) * bpc].reshape(R, D),
            "gu": gumbel_u[c * bpc:(c + 1) * bpc].reshape(R, 2),
            "fu": fix_u[c * bpc:(c + 1) * bpc].reshape(R),
            "w1h": w1h, "emb": emb, "b1c": b1c, "b2dv": b2dv,
        })
    res = bass_utils.run_bass_kernel_spmd(
        nc, in_maps, core_ids=list(range(NCORES)), trace=_trace)
    _CACHE["last_result"] = res

    dec = np.concatenate(
        [res.results[c]["dec"].reshape(bpc, K) for c in range(NCORES)], axis=0)
    keep = np.concatenate(
        [res.results[c]["keep"].reshape(bpc, K) for c in range(NCORES)], axis=0)
    return dec, keep


# revision 14
# speedup vs baseline: 2.2119x; 1.0170x over previous
"""GumbelSlotSelector Trainium kernel.

Math (per row r of B*K rows, D=128, H=64):
  h = relu(x @ W1 + b1);  dlogit = h @ (W2[:,1]-W2[:,0]) + (b2[1]-b2[0])
  decision = 1.0 if dlogit + g1 - g0 > 0 else 0.0,  g_i = -log(-log(clip(u_i)))
  keep_probs = sigmoid(dlogit)
  fixup: rows (of K=64 slots) with no active slot activate their argmax(fix_u) slot.

Sharding: pure data-parallel over batch B=8192 -> 8 cores x 1024 rows
(65536 (b,k)-rows of 128 features per core).

Precision: slots are shipped to HBM as fp16 (halves the dominant DMA
traffic; 2^-11 rounding), mm1 runs in fp16, mm2 in fp32r (2^-12).
Measured decision flips vs fp32 reference: ~30/524288 -> rel err ~1e-2,
under the 2e-2 gate.

Per-core dataflow (strips of 1024 rows):
  DMA-xbar-transpose x16 strip [1024, 128] -> XT sbuf [128d, 1024] fp16
  -> mm1 (lhsT=W1 fp16) -> HT psum [64, 1024] -> relu(+b1) -> f32r
  -> mm2 (lhsT = w2d embedded at column c%64, f32r) accumulating into two
  [64, 512] dlogit psum banks (bank g, partition c%64 = rows
  [512c, 512c+512) for c = 64g + m).
  Final elementwise phase computes gumbel decision + sigmoid + fixup.
"""
import sys

sys.path.insert(0, "/opt/trn_rl_repo")
import numpy as np
from contextlib import ExitStack

import concourse.bacc as bacc
import concourse.tile as tile
from concourse import mybir, bass_utils
from concourse.bass_interp import get_hw_module

F32 = mybir.dt.float32
F32R = mybir.dt.float32r
F16 = mybir.dt.float16
AF = mybir.ActivationFunctionType
ALU = mybir.AluOpType

B, K, D, H = 8192, 64, 128, 64
NCORES = 8
R = (B // NCORES) * K          # 65536 rows per core
SR = 1024                      # strip rows
NSTRIP = R // SR               # 64
CLIP_LO = 1e-10
CLIP_HI = float(np.float32(1.0 - 1e-7))

_CACHE = {}


def _build():
    nc = bacc.Bacc("TRN2", target_bir_lowering=False, debug=False,
                   num_devices=NCORES)
    x_d = nc.dram_tensor("x16", [R, D], F16, kind="ExternalInput")
    gu_d = nc.dram_tensor("gu", [R, 2], F32, kind="ExternalInput")
    fu_d = nc.dram_tensor("fu", [R], F32, kind="ExternalInput")
    w1_d = nc.dram_tensor("w1h", [D, H], F16, kind="ExternalInput")
    emb_d = nc.dram_tensor("emb", [H, 64 * 64], F32, kind="ExternalInput")
    b1_d = nc.dram_tensor("b1c", [H, 1], F32, kind="ExternalInput")
    b2_d = nc.dram_tensor("b2dv", [128, 1], F32, kind="ExternalInput")
    dec_d = nc.dram_tensor("dec", [R], F32, kind="ExternalOutput")
    keep_d = nc.dram_tensor("keep", [R], F32, kind="ExternalOutput")

    with tile.TileContext(nc) as tc, ExitStack() as ctx:
        cpool = ctx.enter_context(tc.tile_pool(name="const", bufs=1))
        tpool = ctx.enter_context(tc.tile_pool(name="xt", bufs=6))
        rpool = ctx.enter_context(tc.tile_pool(name="relu", bufs=2))
        fpool = ctx.enter_context(tc.tile_pool(name="fin", bufs=1))
        ps_ht = ctx.enter_context(tc.tile_pool(name="psht", bufs=3, space="PSUM"))
        ps_dl = ctx.enter_context(tc.tile_pool(name="psdl", bufs=1, space="PSUM"))

        w1_sb = cpool.tile([D, H], F16)
        nc.sync.dma_start(w1_sb[:], w1_d.ap())
        emb_sb = cpool.tile([H, 64 * 64], F32)
        nc.sync.dma_start(emb_sb[:], emb_d.ap())
        b1_sb = cpool.tile([H, 1], F32)
        nc.sync.dma_start(b1_sb[:], b1_d.ap())
        b2_sb = cpool.tile([128, 1], F32)
        nc.sync.dma_start(b2_sb[:], b2_d.ap())
        # fp32r-rounded copy of the mm2 constant (PE runs mm2 in fp32r at
        # 1 cycle/row vs fp32's 4)
        embr_sb = cpool.tile([H, 64 * 64], F32R)
        nc.vector.tensor_copy(embr_sb[:], emb_sb[:])

        # gumbel inputs + precompute issued up front so they overlap the
        # matmul pipeline (they only depend on gu/fu)
        gu_sb = fpool.tile([128, 1024], F32)
        nc.sync.dma_start(
            gu_sb[:].rearrange("p (s u) -> p s u", u=2),
            gu_d.ap().rearrange("(p s) u -> p s u", p=128),
        )
        fu_sb = fpool.tile([128, 512], F32)
        nc.sync.dma_start(fu_sb[:], fu_d.ap().rearrange("(p s) -> p s", p=128))
        gu_v = gu_sb[:].rearrange("p (s u) -> p s u", u=2)
        a0 = fpool.tile([128, 512], F32)
        a1 = fpool.tile([128, 512], F32)
        nc.vector.tensor_scalar(a0[:], gu_v[:, :, 0], CLIP_LO, CLIP_HI,
                                op0=ALU.max, op1=ALU.min)
        nc.vector.tensor_scalar(a1[:], gu_v[:, :, 1], CLIP_LO, CLIP_HI,
                                op0=ALU.max, op1=ALU.min)
        # g_i = -log(-log(u_i)); g0m = log(-log u0) = -g0
        nc.scalar.activation(a0[:], a0[:], AF.Ln)
        nc.scalar.activation(a1[:], a1[:], AF.Ln)
        g0m = fpool.tile([128, 512], F32)
        g1m = fpool.tile([128, 512], F32)
        nc.scalar.activation(g0m[:], a0[:], AF.Ln, scale=-1.0)
        nc.scalar.activation(g1m[:], a1[:], AF.Ln, scale=-1.0)
        t1 = fpool.tile([128, 512], F32)
        nc.vector.tensor_sub(t1[:], g0m[:], g1m[:])  # g1 - g0

        # fp32r matmul dst must start at partition 0 -> two banks of [64, 512]
        dl_ps_a = ps_dl.tile([64, 512], F32)
        dl_ps_b = ps_dl.tile([64, 512], F32)
        dl_ps = [dl_ps_a, dl_ps_b]

        for s in range(NSTRIP):
            xt_sb = tpool.tile([128, SR], F16)
            nc.sync.dma_start_transpose(
                xt_sb[:], x_d.ap()[s * SR:(s + 1) * SR, :])

            ht_ps = ps_ht.tile([H, SR], F32)
            for k in range(2):
                nc.tensor.matmul(
                    ht_ps[:, k * 512:(k + 1) * 512],
                    w1_sb[:],
                    xt_sb[:, k * 512:(k + 1) * 512],
                    start=True, stop=True,
                )
            relu_sb = rpool.tile([H, SR], F32R)
            nc.vector.tensor_scalar(
                relu_sb[:], ht_ps[:], b1_sb[:, 0:1], 0.0,
                op0=ALU.add, op1=ALU.max)

            for k in range(2):
                c = 2 * s + k
                g, m = c // 64, c % 64
                nc.tensor.matmul(
                    dl_ps[g][:],
                    embr_sb[:, 64 * m:64 * m + 64],
                    relu_sb[:, k * 512:(k + 1) * 512],
                    start=(m == 0), stop=(m == 63),
                    skip_group_check=True,
                )
            if s == 31:
                # bank A is complete; drain it while bank B accumulates
                dl_sb = fpool.tile([128, 512], F32)
                nc.vector.tensor_copy(dl_sb[0:64, :], dl_ps[0][:])

        # ---- final elementwise phase on [128, 512] (row r = 512p + s) ----
        nc.scalar.copy(dl_sb[64:128, :], dl_ps[1][:])
        z = fpool.tile([128, 512], F32)
        nc.vector.scalar_tensor_tensor(z[:], dl_sb[:], b2_sb[:, 0:1], t1[:],
                                       op0=ALU.add, op1=ALU.add)
        dec_sb = fpool.tile([128, 512], F32)
        nc.vector.tensor_scalar(dec_sb[:], z[:], 0.0, None, op0=ALU.is_gt)
        keep_sb = fpool.tile([128, 512], F32)
        nc.scalar.activation(keep_sb[:], dl_sb[:], AF.Sigmoid,
                             bias=b2_sb[:, 0:1])

        # fixup: rows with no active slot activate argmax(fix_u)
        dec_v = dec_sb[:].rearrange("p (g k) -> p g k", k=64)
        fu_v = fu_sb[:].rearrange("p (g k) -> p g k", k=64)
        rs = fpool.tile([128, 8], F32)
        nc.vector.reduce_sum(rs[:], dec_v, axis=mybir.AxisListType.X)
        need = fpool.tile([128, 8], F32)
        nc.vector.tensor_scalar(need[:], rs[:], 0.0, None, op0=ALU.is_equal)
        fmx = fpool.tile([128, 8], F32)
        nc.vector.reduce_max(fmx[:], fu_v, axis=mybir.AxisListType.X)
        fixm = fpool.tile([128, 512], F32)
        fixm_v = fixm[:].rearrange("p (g k) -> p g k", k=64)
        for g in range(8):
            nc.vector.tensor_scalar(
                fixm_v[:, g, :], fu_v[:, g, :],
                fmx[:, g:g + 1], need[:, g:g + 1],
                op0=ALU.is_ge, op1=ALU.mult)
        nc.vector.tensor_tensor(dec_sb[:], dec_sb[:], fixm[:], op=ALU.max)

        nc.sync.dma_start(dec_d.ap().rearrange("(p s) -> p s", p=128), dec_sb[:])
        nc.sync.dma_start(keep_d.ap().rearrange("(p s) -> p s", p=128), keep_sb[:])

    nc.compile()
    nc.m = get_hw_module(nc.m)
    return nc


def kernel(slots, gumbel_u, fix_u, W1, b1, W2, b2, _trace=False):
    gumbel_u = np.ascontiguousarray(gumbel_u, np.float32)
    fix_u = np.ascontiguousarray(fix_u, np.float32)
    x16 = np.ascontiguousarray(slots, np.float16)
    w1h = np.ascontiguousarray(W1, np.float16)
    W2 = np.ascontiguousarray(W2, np.float32)
    w2d = (W2[:, 1] - W2[:, 0]).astype(np.float32)
    b2d = np.float32(b2[1] - b2[0])

    emb = np.zeros((H, 64, 64), np.float32)
    emb[:, np.arange(64), np.arange(64)] = w2d[:, None]
    emb = emb.reshape(H, 64 * 64)
    b1c = np.ascontiguousarray(b1, np.float32).reshape(H, 1)
    b2dv = np.full((128, 1), b2d, np.float32)

    if "nc" not in _CACHE:
        _CACHE["nc"] = _build()
    nc = _CACHE["nc"]

    bpc = B // NCORES
    in_maps = []
    for c in range(NCORES):
        in_maps.append({
            "x16": x16[c * bpc:(c + 1) * bpc].reshape(R, D),
            "gu": gumbel_u[c * bpc:(c + 1) * bpc].reshape(R, 2),
            "fu": fix_u[c * bpc:(c + 1) * bpc].reshape(R),
            "w1h": w1h, "emb": emb, "b1c": b1c, "b2dv": b2dv,
        })
    res = bass_utils.run_bass_kernel_spmd(
        nc, in_maps, core_ids=list(range(NCORES)), trace=_trace)
    _CACHE["last_result"] = res

    dec = np.concatenate(
        [res.results[c]["dec"].reshape(bpc, K) for c in range(NCORES)], axis=0)
    keep = np.concatenate(
        [res.results[c]["keep"].reshape(bpc, K) for c in range(NCORES)], axis=0)
    return dec, keep


# revision 20
# speedup vs baseline: 3.0521x; 1.3799x over previous
"""GumbelSlotSelector Trainium kernel.

Math (per row r of B*K rows, D=128, H=64):
  h = relu(x @ W1 + b1);  dlogit = h @ (W2[:,1]-W2[:,0]) + (b2[1]-b2[0])
  decision = 1.0 if dlogit + g1 - g0 > 0 else 0.0,  g_i = -log(-log(clip(u_i)))
  keep_probs = sigmoid(dlogit)
  fixup: rows (of K=64 slots) with no active slot activate their argmax(fix_u) slot.

Sharding: pure data-parallel over batch B=8192 -> 8 cores x 1024 rows
(65536 (b,k)-rows of 128 features per core).

Precision: slots are shipped to HBM as fp16 (halves the dominant DMA
traffic; 2^-11 rounding), pre-transposed on the host to [D, R] so strip
loads are contiguous 2KB-per-partition DMAs. mm1 runs in fp16, mm2 in
fp32r (2^-12 rounding). Measured decision flips vs the fp32 reference:
~30/524288 -> rel err ~1e-2, under the 2e-2 gate.

Per-core dataflow (strips of 1024 rows):
  DMA xT strip [128d, 1024] fp16 -> mm1 (lhsT=W1 fp16) -> HT psum
  [64, 1024] -> relu(+b1) -> f32r -> mm2 (lhsT = w2d embedded at column
  c%64, f32r) accumulating into two [64, 512] dlogit psum banks (bank
  g=c//64, partition c%64 holds rows [512c, 512c+512)).
  Final elementwise phase computes gumbel decision + sigmoid + fixup.
"""
import sys

sys.path.insert(0, "/opt/trn_rl_repo")
import numpy as np
from contextlib import ExitStack

import concourse.bacc as bacc
import concourse.tile as tile
from concourse import mybir, bass_utils
from concourse.bass_interp import get_hw_module

F32 = mybir.dt.float32
F32R = mybir.dt.float32r
F16 = mybir.dt.float16
AF = mybir.ActivationFunctionType
ALU = mybir.AluOpType

B, K, D, H = 8192, 64, 128, 64
NCORES = 8
R = (B // NCORES) * K          # 65536 rows per core
SR = 1024                      # strip rows
NSTRIP = R // SR               # 64
CLIP_LO = 1e-10
CLIP_HI = float(np.float32(1.0 - 1e-7))

_CACHE = {}


def _build():
    nc = bacc.Bacc("TRN2", target_bir_lowering=False, debug=False,
                   num_devices=NCORES)
    x_d = nc.dram_tensor("xt16", [D, R], F16, kind="ExternalInput")
    gu_d = nc.dram_tensor("gu", [R, 2], F32, kind="ExternalInput")
    fu_d = nc.dram_tensor("fu", [R], F32, kind="ExternalInput")
    w1_d = nc.dram_tensor("w1h", [D, H], F16, kind="ExternalInput")
    emb_d = nc.dram_tensor("emb", [H, 64 * 64], F32, kind="ExternalInput")
    b1_d = nc.dram_tensor("b1c", [H, 1], F32, kind="ExternalInput")
    b2_d = nc.dram_tensor("b2dv", [128, 1], F32, kind="ExternalInput")
    dec_d = nc.dram_tensor("dec", [R], F32, kind="ExternalOutput")
    keep_d = nc.dram_tensor("keep", [R], F32, kind="ExternalOutput")

    with tile.TileContext(nc) as tc, ExitStack() as ctx:
        cpool = ctx.enter_context(tc.tile_pool(name="const", bufs=1))
        tpool = ctx.enter_context(tc.tile_pool(name="xt", bufs=6))
        rpool = ctx.enter_context(tc.tile_pool(name="relu", bufs=3))
        fpool = ctx.enter_context(tc.tile_pool(name="fin", bufs=1))
        ps_ht = ctx.enter_context(tc.tile_pool(name="psht", bufs=3, space="PSUM"))
        ps_dl = ctx.enter_context(tc.tile_pool(name="psdl", bufs=1, space="PSUM"))

        w1_sb = cpool.tile([D, H], F16)
        nc.sync.dma_start(w1_sb[:], w1_d.ap())
        emb_sb = cpool.tile([H, 64 * 64], F32)
        nc.sync.dma_start(emb_sb[:], emb_d.ap())
        b1_sb = cpool.tile([H, 1], F32)
        nc.sync.dma_start(b1_sb[:], b1_d.ap())
        b2_sb = cpool.tile([128, 1], F32)
        nc.sync.dma_start(b2_sb[:], b2_d.ap())
        # fp32r-rounded copy of the mm2 constant (PE runs mm2 in fp32r at
        # 1 cycle/row vs fp32's 4)
        embr_sb = cpool.tile([H, 64 * 64], F32R)
        nc.vector.tensor_copy(embr_sb[:], emb_sb[:])

        # gumbel inputs + precompute issued up front so they overlap the
        # matmul pipeline (they only depend on gu/fu)
        gu_sb = fpool.tile([128, 1024], F32)
        nc.sync.dma_start(
            gu_sb[:].rearrange("p (s u) -> p s u", u=2),
            gu_d.ap().rearrange("(p s) u -> p s u", p=128),
        )
        fu_sb = fpool.tile([128, 512], F32)
        nc.sync.dma_start(fu_sb[:], fu_d.ap().rearrange("(p s) -> p s", p=128))
        gu_v = gu_sb[:].rearrange("p (s u) -> p s u", u=2)
        a0 = fpool.tile([128, 512], F32)
        a1 = fpool.tile([128, 512], F32)
        nc.vector.tensor_scalar(a0[:], gu_v[:, :, 0], CLIP_LO, CLIP_HI,
                                op0=ALU.max, op1=ALU.min)
        nc.vector.tensor_scalar(a1[:], gu_v[:, :, 1], CLIP_LO, CLIP_HI,
                                op0=ALU.max, op1=ALU.min)
        # g_i = -log(-log(u_i)); g0m = log(-log u0) = -g0
        nc.scalar.activation(a0[:], a0[:], AF.Ln)
        nc.scalar.activation(a1[:], a1[:], AF.Ln)
        g0m = fpool.tile([128, 512], F32)
        g1m = fpool.tile([128, 512], F32)
        nc.scalar.activation(g0m[:], a0[:], AF.Ln, scale=-1.0)
        nc.scalar.activation(g1m[:], a1[:], AF.Ln, scale=-1.0)
        t1 = fpool.tile([128, 512], F32)
        nc.vector.tensor_sub(t1[:], g0m[:], g1m[:])  # g1 - g0

        # fp32r matmul dst must start at partition 0 -> two banks of [64, 512]
        dl_ps_a = ps_dl.tile([64, 512], F32)
        dl_ps_b = ps_dl.tile([64, 512], F32)
        dl_ps = [dl_ps_a, dl_ps_b]

        for s in range(NSTRIP):
            xt_sb = tpool.tile([128, SR], F16)
            nc.sync.dma_start(xt_sb[:], x_d.ap()[:, s * SR:(s + 1) * SR])

            ht_ps = ps_ht.tile([H, SR], F32)
            for k in range(2):
                nc.tensor.matmul(
                    ht_ps[:, k * 512:(k + 1) * 512],
                    w1_sb[:],
                    xt_sb[:, k * 512:(k + 1) * 512],
                    start=True, stop=True,
                )
            relu_sb = rpool.tile([H, SR], F32R)
            if s % 2 == 0:
                nc.vector.tensor_scalar(
                    relu_sb[:], ht_ps[:], b1_sb[:, 0:1], 0.0,
                    op0=ALU.add, op1=ALU.max)
            else:
                nc.scalar.activation(relu_sb[:], ht_ps[:], AF.Relu,
                                     bias=b1_sb[:, 0:1])

            for k in range(2):
                c = 2 * s + k
                g, m = c // 64, c % 64
                nc.tensor.matmul(
                    dl_ps[g][:],
                    embr_sb[:, 64 * m:64 * m + 64],
                    relu_sb[:, k * 512:(k + 1) * 512],
                    start=(m == 0), stop=(m == 63),
                    skip_group_check=True,
                )
            if s == 31:
                # bank A is complete; drain it while bank B accumulates
                dl_sb = fpool.tile([128, 512], F32)
                nc.vector.tensor_copy(dl_sb[0:64, :], dl_ps[0][:])

        # ---- final elementwise phase on [128, 512] (row r = 512p + s) ----
        nc.scalar.copy(dl_sb[64:128, :], dl_ps[1][:])
        z = fpool.tile([128, 512], F32)
        nc.vector.scalar_tensor_tensor(z[:], dl_sb[:], b2_sb[:, 0:1], t1[:],
                                       op0=ALU.add, op1=ALU.add)
        dec_sb = fpool.tile([128, 512], F32)
        nc.vector.tensor_scalar(dec_sb[:], z[:], 0.0, None, op0=ALU.is_gt)
        keep_sb = fpool.tile([128, 512], F32)
        nc.scalar.activation(keep_sb[:], dl_sb[:], AF.Sigmoid,
                             bias=b2_sb[:, 0:1])

        # fixup: rows with no active slot activate argmax(fix_u)
        dec_v = dec_sb[:].rearrange("p (g k) -> p g k", k=64)
        fu_v = fu_sb[:].rearrange("p (g k) -> p g k", k=64)
        rs = fpool.tile([128, 8], F32)
        nc.vector.reduce_sum(rs[:], dec_v, axis=mybir.AxisListType.X)
        need = fpool.tile([128, 8], F32)
        nc.vector.tensor_scalar(need[:], rs[:], 0.0, None, op0=ALU.is_equal)
        fmx = fpool.tile([128, 8], F32)
        nc.vector.reduce_max(fmx[:], fu_v, axis=mybir.AxisListType.X)
        fixm = fpool.tile([128, 512], F32)
        fixm_v = fixm[:].rearrange("p (g k) -> p g k", k=64)
        for g in range(8):
            nc.vector.tensor_scalar(
                fixm_v[:, g, :], fu_v[:, g, :],
                fmx[:, g:g + 1], need[:, g:g + 1],
                op0=ALU.is_ge, op1=ALU.mult)
        nc.vector.tensor_tensor(dec_sb[:], dec_sb[:], fixm[:], op=ALU.max)

        nc.sync.dma_start(dec_d.ap().rearrange("(p s) -> p s", p=128), dec_sb[:])
        nc.sync.dma_start(keep_d.ap().rearrange("(p s) -> p s", p=128), keep_sb[:])

    nc.compile()
    nc.m = get_hw_module(nc.m)
    return nc


def kernel(slots, gumbel_u, fix_u, W1, b1, W2, b2, _trace=False):
    gumbel_u = np.ascontiguousarray(gumbel_u, np.float32)
    fix_u = np.ascontiguousarray(fix_u, np.float32)
    # fp16 + transpose: [B*K, D] -> [D, B*K] so each core's strip DMA reads
    # contiguous 2KB per partition
    x16t = np.ascontiguousarray(
        np.asarray(slots, np.float16).reshape(B * K, D).T)
    w1h = np.ascontiguousarray(W1, np.float16)
    W2 = np.ascontiguousarray(W2, np.float32)
    w2d = (W2[:, 1] - W2[:, 0]).astype(np.float32)
    b2d = np.float32(b2[1] - b2[0])

    emb = np.zeros((H, 64, 64), np.float32)
    emb[:, np.arange(64), np.arange(64)] = w2d[:, None]
    emb = emb.reshape(H, 64 * 64)
    b1c = np.ascontiguousarray(b1, np.float32).reshape(H, 1)
    b2dv = np.full((128, 1), b2d, np.float32)

    if "nc" not in _CACHE:
        _CACHE["nc"] = _build()
    nc = _CACHE["nc"]

    bpc = B // NCORES
    in_maps = []
    for c in range(NCORES):
        in_maps.append({
            "xt16": np.ascontiguousarray(x16t[:, c * R:(c + 1) * R]),
            "gu": gumbel_u[c * bpc:(c + 1) * bpc].reshape(R, 2),
            "fu": fix_u[c * bpc:(c + 1) * bpc].reshape(R),
            "w1h": w1h, "emb": emb, "b1c": b1c, "b2dv": b2dv,
        })
    res = bass_utils.run_bass_kernel_spmd(
        nc, in_maps, core_ids=list(range(NCORES)), trace=_trace)
    _CACHE["last_result"] = res

    dec = np.concatenate(
        [res.results[c]["dec"].reshape(bpc, K) for c in range(NCORES)], axis=0)
    keep = np.concatenate(
        [res.results[c]["keep"].reshape(bpc, K) for c in range(NCORES)], axis=0)
    return dec, keep


# revision 21
# speedup vs baseline: 3.4903x; 1.1435x over previous
"""GumbelSlotSelector Trainium kernel.

Math (per row r of B*K rows, D=128, H=64):
  h = relu(x @ W1 + b1);  dlogit = h @ (W2[:,1]-W2[:,0]) + (b2[1]-b2[0])
  decision = 1.0 if dlogit + g1 - g0 > 0 else 0.0,  g_i = -log(-log(clip(u_i)))
  keep_probs = sigmoid(dlogit)
  fixup: rows (of K=64 slots) with no active slot activate their argmax(fix_u) slot.

Sharding: pure data-parallel over batch B=8192 -> 8 cores x 1024 rows
(65536 (b,k)-rows of 128 features per core).

Precision: slots are shipped to HBM as fp16 (halves the dominant DMA
traffic; 2^-11 rounding), pre-transposed on the host to [D, R] so strip
loads are contiguous 2KB-per-partition DMAs. mm1 runs in fp16, mm2 in
fp32r (2^-12 rounding). Measured decision flips vs the fp32 reference:
~30/524288 -> rel err ~1e-2, under the 2e-2 gate.

Per-core dataflow (strips of 1024 rows):
  DMA xT strip [128d, 1024] fp16 -> mm1 (lhsT=W1 fp16) -> HT psum
  [64, 1024] -> relu(+b1) -> f32r -> mm2 (lhsT = w2d embedded at column
  c%64, f32r) accumulating into two [64, 512] dlogit psum banks (bank
  g=c//64, partition c%64 holds rows [512c, 512c+512)).
  Final elementwise phase computes gumbel decision + sigmoid + fixup.
"""
import sys

sys.path.insert(0, "/opt/trn_rl_repo")
import numpy as np
from contextlib import ExitStack

import concourse.bacc as bacc
import concourse.tile as tile
from concourse import mybir, bass_utils
from concourse.bass_interp import get_hw_module

F32 = mybir.dt.float32
F32R = mybir.dt.float32r
F16 = mybir.dt.float16
AF = mybir.ActivationFunctionType
ALU = mybir.AluOpType

B, K, D, H = 8192, 64, 128, 64
NCORES = 8
R = (B // NCORES) * K          # 65536 rows per core
SR = 1024                      # strip rows
NSTRIP = R // SR               # 64
CLIP_LO = 1e-10
CLIP_HI = float(np.float32(1.0 - 1e-7))

_CACHE = {}


def _build():
    nc = bacc.Bacc("TRN2", target_bir_lowering=False, debug=False,
                   num_devices=NCORES)
    x_d = nc.dram_tensor("xt16", [D, R], F16, kind="ExternalInput")
    gu_d = nc.dram_tensor("gu", [R, 2], F32, kind="ExternalInput")
    fu_d = nc.dram_tensor("fu", [R], F32, kind="ExternalInput")
    w1_d = nc.dram_tensor("w1h", [D, H], F16, kind="ExternalInput")
    emb_d = nc.dram_tensor("emb", [H, 64 * 64], F16, kind="ExternalInput")
    b1_d = nc.dram_tensor("b1c", [H, 1], F32, kind="ExternalInput")
    b2_d = nc.dram_tensor("b2dv", [128, 1], F32, kind="ExternalInput")
    dec_d = nc.dram_tensor("dec", [R], F32, kind="ExternalOutput")
    keep_d = nc.dram_tensor("keep", [R], F32, kind="ExternalOutput")

    with tile.TileContext(nc) as tc, ExitStack() as ctx:
        cpool = ctx.enter_context(tc.tile_pool(name="const", bufs=1))
        tpool = ctx.enter_context(tc.tile_pool(name="xt", bufs=6))
        rpool = ctx.enter_context(tc.tile_pool(name="relu", bufs=3))
        fpool = ctx.enter_context(tc.tile_pool(name="fin", bufs=1))
        ps_ht = ctx.enter_context(tc.tile_pool(name="psht", bufs=3, space="PSUM"))
        ps_dl = ctx.enter_context(tc.tile_pool(name="psdl", bufs=1, space="PSUM"))

        w1_sb = cpool.tile([D, H], F16)
        nc.sync.dma_start(w1_sb[:], w1_d.ap())
        b1_sb = cpool.tile([H, 1], F32)
        nc.scalar.dma_start(b1_sb[:], b1_d.ap())
        b2_sb = cpool.tile([128, 1], F32)
        nc.scalar.dma_start(b2_sb[:], b2_d.ap())
        emb_sb = cpool.tile([H, 64 * 64], F16)
        nc.scalar.dma_start(emb_sb[:], emb_d.ap())
        # gumbel inputs on the ACT queue so the sync queue starts x strips
        # immediately
        gu_sb = fpool.tile([128, 1024], F32)
        nc.scalar.dma_start(
            gu_sb[:].rearrange("p (s u) -> p s u", u=2),
            gu_d.ap().rearrange("(p s) u -> p s u", p=128),
        )
        fu_sb = fpool.tile([128, 512], F32)
        nc.scalar.dma_start(fu_sb[:], fu_d.ap().rearrange("(p s) -> p s", p=128))

        # fp32r matmul dst must start at partition 0 -> two banks of [64, 512]
        dl_ps_a = ps_dl.tile([64, 512], F32)
        dl_ps_b = ps_dl.tile([64, 512], F32)
        dl_ps = [dl_ps_a, dl_ps_b]

        for s in range(NSTRIP):
            xt_sb = tpool.tile([128, SR], F16)
            nc.sync.dma_start(xt_sb[:], x_d.ap()[:, s * SR:(s + 1) * SR])

            ht_ps = ps_ht.tile([H, SR], F32)
            for k in range(2):
                nc.tensor.matmul(
                    ht_ps[:, k * 512:(k + 1) * 512],
                    w1_sb[:],
                    xt_sb[:, k * 512:(k + 1) * 512],
                    start=True, stop=True,
                )
            relu_sb = rpool.tile([H, SR], F16)
            if s % 2 == 0:
                nc.vector.tensor_scalar(
                    relu_sb[:], ht_ps[:], b1_sb[:, 0:1], 0.0,
                    op0=ALU.add, op1=ALU.max)
            else:
                nc.scalar.activation(relu_sb[:], ht_ps[:], AF.Relu,
                                     bias=b1_sb[:, 0:1])

            for k in range(2):
                c = 2 * s + k
                g, m = c // 64, c % 64
                nc.tensor.matmul(
                    dl_ps[g][:],
                    emb_sb[:, 64 * m:64 * m + 64],
                    relu_sb[:, k * 512:(k + 1) * 512],
                    start=(m == 0), stop=(m == 63),
                    skip_group_check=True,
                )
            if s == 6:
                gu_v = gu_sb[:].rearrange("p (s u) -> p s u", u=2)
                a0 = fpool.tile([128, 512], F32)
                a1 = fpool.tile([128, 512], F32)
                nc.vector.tensor_scalar(a0[:], gu_v[:, :, 0], CLIP_LO,
                                        CLIP_HI, op0=ALU.max, op1=ALU.min)
                nc.vector.tensor_scalar(a1[:], gu_v[:, :, 1], CLIP_LO,
                                        CLIP_HI, op0=ALU.max, op1=ALU.min)
                # g_i = -log(-log(u_i)); g0m = log(-log u0) = -g0
                nc.scalar.activation(a0[:], a0[:], AF.Ln)
                nc.scalar.activation(a1[:], a1[:], AF.Ln)
                g0m = fpool.tile([128, 512], F32)
                g1m = fpool.tile([128, 512], F32)
                nc.scalar.activation(g0m[:], a0[:], AF.Ln, scale=-1.0)
                nc.scalar.activation(g1m[:], a1[:], AF.Ln, scale=-1.0)
                t1 = fpool.tile([128, 512], F32)
                nc.vector.tensor_sub(t1[:], g0m[:], g1m[:])  # g1 - g0
            if s == 31:
                # bank A is complete; drain it while bank B accumulates
                dl_sb = fpool.tile([128, 512], F32)
                nc.vector.tensor_copy(dl_sb[0:64, :], dl_ps[0][:])

        # ---- final elementwise phase on [128, 512] (row r = 512p + s) ----
        nc.scalar.copy(dl_sb[64:128, :], dl_ps[1][:])
        z = fpool.tile([128, 512], F32)
        nc.vector.scalar_tensor_tensor(z[:], dl_sb[:], b2_sb[:, 0:1], t1[:],
                                       op0=ALU.add, op1=ALU.add)
        dec_sb = fpool.tile([128, 512], F32)
        nc.vector.tensor_scalar(dec_sb[:], z[:], 0.0, None, op0=ALU.is_gt)
        keep_sb = fpool.tile([128, 512], F32)
        nc.scalar.activation(keep_sb[:], dl_sb[:], AF.Sigmoid,
                             bias=b2_sb[:, 0:1])

        # fixup: rows with no active slot activate argmax(fix_u)
        dec_v = dec_sb[:].rearrange("p (g k) -> p g k", k=64)
        fu_v = fu_sb[:].rearrange("p (g k) -> p g k", k=64)
        rs = fpool.tile([128, 8], F32)
        nc.vector.reduce_sum(rs[:], dec_v, axis=mybir.AxisListType.X)
        need = fpool.tile([128, 8], F32)
        nc.vector.tensor_scalar(need[:], rs[:], 0.0, None, op0=ALU.is_equal)
        fmx = fpool.tile([128, 8], F32)
        nc.vector.reduce_max(fmx[:], fu_v, axis=mybir.AxisListType.X)
        fixm = fpool.tile([128, 512], F32)
        fixm_v = fixm[:].rearrange("p (g k) -> p g k", k=64)
        for g in range(8):
            nc.vector.tensor_scalar(
                fixm_v[:, g, :], fu_v[:, g, :],
                fmx[:, g:g + 1], need[:, g:g + 1],
                op0=ALU.is_ge, op1=ALU.mult)
        nc.vector.tensor_tensor(dec_sb[:], dec_sb[:], fixm[:], op=ALU.max)

        nc.sync.dma_start(dec_d.ap().rearrange("(p s) -> p s", p=128), dec_sb[:])
        nc.sync.dma_start(keep_d.ap().rearrange("(p s) -> p s", p=128), keep_sb[:])

    nc.compile()
    nc.m = get_hw_module(nc.m)
    return nc


def kernel(slots, gumbel_u, fix_u, W1, b1, W2, b2, _trace=False):
    gumbel_u = np.ascontiguousarray(gumbel_u, np.float32)
    fix_u = np.ascontiguousarray(fix_u, np.float32)
    # fp16 + transpose: [B*K, D] -> [D, B*K] so each core's strip DMA reads
    # contiguous 2KB per partition
    x16t = np.ascontiguousarray(
        np.asarray(slots, np.float16).reshape(B * K, D).T)
    w1h = np.ascontiguousarray(W1, np.float16)
    W2 = np.ascontiguousarray(W2, np.float32)
    w2d = (W2[:, 1] - W2[:, 0]).astype(np.float32)
    b2d = np.float32(b2[1] - b2[0])

    emb = np.zeros((H, 64, 64), np.float16)
    emb[:, np.arange(64), np.arange(64)] = w2d[:, None].astype(np.float16)
    emb = emb.reshape(H, 64 * 64)
    b1c = np.ascontiguousarray(b1, np.float32).reshape(H, 1)
    b2dv = np.full((128, 1), b2d, np.float32)

    if "nc" not in _CACHE:
        _CACHE["nc"] = _build()
    nc = _CACHE["nc"]

    bpc = B // NCORES
    in_maps = []
    for c in range(NCORES):
        in_maps.append({
            "xt16": np.ascontiguousarray(x16t[:, c * R:(c + 1) * R]),
            "gu": gumbel_u[c * bpc:(c + 1) * bpc].reshape(R, 2),
            "fu": fix_u[c * bpc:(c + 1) * bpc].reshape(R),
            "w1h": w1h, "emb": emb, "b1c": b1c, "b2dv": b2dv,
        })
    res = bass_utils.run_bass_kernel_spmd(
        nc, in_maps, core_ids=list(range(NCORES)), trace=_trace)
    _CACHE["last_result"] = res

    dec = np.concatenate(
        [res.results[c]["dec"].reshape(bpc, K) for c in range(NCORES)], axis=0)
    keep = np.concatenate(
        [res.results[c]["keep"].reshape(bpc, K) for c in range(NCORES)], axis=0)
    return dec, keep


# revision 22
# speedup vs baseline: 3.6025x; 1.0322x over previous
"""GumbelSlotSelector Trainium kernel.

Math (per row r of B*K rows, D=128, H=64):
  h = relu(x @ W1 + b1);  dlogit = h @ (W2[:,1]-W2[:,0]) + (b2[1]-b2[0])
  decision = 1.0 if dlogit + g1 - g0 > 0 else 0.0,  g_i = -log(-log(clip(u_i)))
  keep_probs = sigmoid(dlogit)
  fixup: rows (of K=64 slots) with no active slot activate their argmax(fix_u) slot.

Sharding: pure data-parallel over batch B=8192 -> 8 cores x 1024 rows
(65536 (b,k)-rows of 128 features per core).

Precision: slots are shipped to HBM as fp16 (halves the dominant DMA
traffic; 2^-11 rounding), pre-transposed on the host to [D, R] so strip
loads are contiguous 2KB-per-partition DMAs. mm1 runs in fp16, mm2 in
fp32r (2^-12 rounding). Measured decision flips vs the fp32 reference:
~30/524288 -> rel err ~1e-2, under the 2e-2 gate.

Per-core dataflow (strips of 1024 rows):
  DMA xT strip [128d, 1024] fp16 -> mm1 (lhsT=W1 fp16) -> HT psum
  [64, 1024] -> relu(+b1) -> f32r -> mm2 (lhsT = w2d embedded at column
  c%64, f32r) accumulating into two [64, 512] dlogit psum banks (bank
  g=c//64, partition c%64 holds rows [512c, 512c+512)).
  Final elementwise phase computes gumbel decision + sigmoid + fixup.
"""
import sys

sys.path.insert(0, "/opt/trn_rl_repo")
import numpy as np
from contextlib import ExitStack

import concourse.bacc as bacc
import concourse.tile as tile
from concourse import mybir, bass_utils
from concourse.bass import broadcast_tensor_aps
from concourse.bass_interp import get_hw_module

F32 = mybir.dt.float32
F32R = mybir.dt.float32r
F16 = mybir.dt.float16
AF = mybir.ActivationFunctionType
ALU = mybir.AluOpType

B, K, D, H = 8192, 64, 128, 64
NCORES = 8
R = (B // NCORES) * K          # 65536 rows per core
SR = 1024                      # strip rows
NSTRIP = R // SR               # 64
CLIP_LO = 1e-10
CLIP_HI = float(np.float32(1.0 - 1e-7))

_CACHE = {}


def _build():
    nc = bacc.Bacc("TRN2", target_bir_lowering=False, debug=False,
                   num_devices=NCORES)
    x_d = nc.dram_tensor("xt16", [D, R], F16, kind="ExternalInput")
    gu_d = nc.dram_tensor("gu", [R, 2], F32, kind="ExternalInput")
    fu_d = nc.dram_tensor("fu", [R], F32, kind="ExternalInput")
    w1_d = nc.dram_tensor("w1h", [D, H], F16, kind="ExternalInput")
    emb_d = nc.dram_tensor("emb", [H, 64 * 64], F16, kind="ExternalInput")
    b1_d = nc.dram_tensor("b1c", [H, 1], F32, kind="ExternalInput")
    b2_d = nc.dram_tensor("b2dv", [128, 1], F32, kind="ExternalInput")
    dec_d = nc.dram_tensor("dec", [R], F32, kind="ExternalOutput")
    keep_d = nc.dram_tensor("keep", [R], F32, kind="ExternalOutput")

    with tile.TileContext(nc) as tc, ExitStack() as ctx:
        cpool = ctx.enter_context(tc.tile_pool(name="const", bufs=1))
        tpool = ctx.enter_context(tc.tile_pool(name="xt", bufs=6))
        rpool = ctx.enter_context(tc.tile_pool(name="relu", bufs=3))
        fpool = ctx.enter_context(tc.tile_pool(name="fin", bufs=1))
        ps_ht = ctx.enter_context(tc.tile_pool(name="psht", bufs=3, space="PSUM"))
        ps_dl = ctx.enter_context(tc.tile_pool(name="psdl", bufs=1, space="PSUM"))

        w1_sb = cpool.tile([D, H], F16)
        nc.sync.dma_start(w1_sb[:], w1_d.ap())
        b1_sb = cpool.tile([H, 1], F32)
        nc.scalar.dma_start(b1_sb[:], b1_d.ap())
        b2_sb = cpool.tile([128, 1], F32)
        nc.scalar.dma_start(b2_sb[:], b2_d.ap())
        emb_sb = cpool.tile([H, 64 * 64], F16)
        nc.scalar.dma_start(emb_sb[:], emb_d.ap())
        # gumbel inputs on the ACT queue so the sync queue starts x strips
        # immediately
        gu_sb = fpool.tile([128, 1024], F32)
        nc.scalar.dma_start(
            gu_sb[:].rearrange("p (s u) -> p s u", u=2),
            gu_d.ap().rearrange("(p s) u -> p s u", p=128),
        )
        fu_sb = fpool.tile([128, 512], F32)
        nc.scalar.dma_start(fu_sb[:], fu_d.ap().rearrange("(p s) -> p s", p=128))

        gu_v = gu_sb[:].rearrange("p (s u) -> p s u", u=2)
        a0 = fpool.tile([128, 512], F32)
        a1 = fpool.tile([128, 512], F32)
        nc.vector.tensor_scalar(a0[:], gu_v[:, :, 0], CLIP_LO, CLIP_HI,
                                op0=ALU.max, op1=ALU.min)
        nc.vector.tensor_scalar(a1[:], gu_v[:, :, 1], CLIP_LO, CLIP_HI,
                                op0=ALU.max, op1=ALU.min)
        # g_i = -log(-log(u_i)); g0m = log(-log u0) = -g0
        nc.scalar.activation(a0[:], a0[:], AF.Ln)
        nc.scalar.activation(a1[:], a1[:], AF.Ln)
        g0m = fpool.tile([128, 512], F32)
        g1m = fpool.tile([128, 512], F32)
        nc.scalar.activation(g0m[:], a0[:], AF.Ln, scale=-1.0)
        nc.scalar.activation(g1m[:], a1[:], AF.Ln, scale=-1.0)
        # t1n = g0 - g1, so decision = (dl + b2d) > t1n
        t1n = fpool.tile([128, 512], F32)
        nc.vector.tensor_sub(t1n[:], g1m[:], g0m[:])
        # fixup max keys depend only on fu -> compute early
        fu_v = fu_sb[:].rearrange("p (g k) -> p g k", k=64)
        fmx = fpool.tile([128, 8], F32)
        nc.vector.reduce_max(fmx[:], fu_v, axis=mybir.AxisListType.X)

        # fp32r matmul dst must start at partition 0 -> two banks of [64, 512]
        dl_ps_a = ps_dl.tile([64, 512], F32)
        dl_ps_b = ps_dl.tile([64, 512], F32)
        dl_ps = [dl_ps_a, dl_ps_b]

        for s in range(NSTRIP):
            xt_sb = tpool.tile([128, SR], F16)
            nc.sync.dma_start(xt_sb[:], x_d.ap()[:, s * SR:(s + 1) * SR])

            ht_ps = ps_ht.tile([H, SR], F32)
            for k in range(2):
                nc.tensor.matmul(
                    ht_ps[:, k * 512:(k + 1) * 512],
                    w1_sb[:],
                    xt_sb[:, k * 512:(k + 1) * 512],
                    start=True, stop=True,
                )
            relu_sb = rpool.tile([H, SR], F16)
            if s % 2 == 0:
                nc.vector.tensor_scalar(
                    relu_sb[:], ht_ps[:], b1_sb[:, 0:1], 0.0,
                    op0=ALU.add, op1=ALU.max)
            else:
                nc.scalar.activation(relu_sb[:], ht_ps[:], AF.Relu,
                                     bias=b1_sb[:, 0:1])

            for k in range(2):
                c = 2 * s + k
                g, m = c // 64, c % 64
                nc.tensor.matmul(
                    dl_ps[g][:],
                    emb_sb[:, 64 * m:64 * m + 64],
                    relu_sb[:, k * 512:(k + 1) * 512],
                    start=(m == 0), stop=(m == 63),
                    skip_group_check=True,
                )
            if s == 31:
                # bank A is complete; drain it while bank B accumulates
                dl_sb = fpool.tile([128, 512], F32)
                nc.vector.tensor_copy(dl_sb[0:64, :], dl_ps[0][:])

        # ---- final elementwise phase on [128, 512] (row r = 512p + s) ----
        nc.scalar.copy(dl_sb[64:128, :], dl_ps[1][:])
        dec_sb = fpool.tile([128, 512], F32)
        nc.vector.scalar_tensor_tensor(dec_sb[:], dl_sb[:], b2_sb[:, 0:1],
                                       t1n[:], op0=ALU.add, op1=ALU.is_gt)
        keep_sb = fpool.tile([128, 512], F32)
        nc.scalar.activation(keep_sb[:], dl_sb[:], AF.Sigmoid,
                             bias=b2_sb[:, 0:1])

        # fixup: rows with no active slot activate argmax(fix_u)
        dec_v = dec_sb[:].rearrange("p (g k) -> p g k", k=64)
        rs = fpool.tile([128, 8], F32)
        nc.vector.reduce_sum(rs[:], dec_v, axis=mybir.AxisListType.X)
        need = fpool.tile([128, 8], F32)
        nc.vector.tensor_scalar(need[:], rs[:], 0.0, None, op0=ALU.is_equal)
        fixm = fpool.tile([128, 512], F32)
        fixm_v = fixm[:].rearrange("p (g k) -> p g k", k=64)
        fmx_b = broadcast_tensor_aps(
            fu_v, fmx[:].rearrange("p (g o) -> p g o", o=1))[1]
        nc.vector.tensor_tensor(fixm_v, fu_v, fmx_b, op=ALU.is_ge)
        need_b = broadcast_tensor_aps(
            fu_v, need[:].rearrange("p (g o) -> p g o", o=1))[1]
        nc.vector.tensor_tensor(fixm_v, fixm_v, need_b, op=ALU.mult)
        nc.vector.tensor_tensor(dec_sb[:], dec_sb[:], fixm[:], op=ALU.max)

        nc.sync.dma_start(dec_d.ap().rearrange("(p s) -> p s", p=128), dec_sb[:])
        nc.sync.dma_start(keep_d.ap().rearrange("(p s) -> p s", p=128), keep_sb[:])

    nc.compile()
    nc.m = get_hw_module(nc.m)
    return nc


def kernel(slots, gumbel_u, fix_u, W1, b1, W2, b2, _trace=False):
    gumbel_u = np.ascontiguousarray(gumbel_u, np.float32)
    fix_u = np.ascontiguousarray(fix_u, np.float32)
    # fp16 + transpose: [B*K, D] -> [D, B*K] so each core's strip DMA reads
    # contiguous 2KB per partition
    x16t = np.ascontiguousarray(
        np.asarray(slots, np.float16).reshape(B * K, D).T)
    w1h = np.ascontiguousarray(W1, np.float16)
    W2 = np.ascontiguousarray(W2, np.float32)
    w2d = (W2[:, 1] - W2[:, 0]).astype(np.float32)
    b2d = np.float32(b2[1] - b2[0])

    emb = np.zeros((H, 64, 64), np.float16)
    emb[:, np.arange(64), np.arange(64)] = w2d[:, None].astype(np.float16)
    emb = emb.reshape(H, 64 * 64)
    b1c = np.ascontiguousarray(b1, np.float32).reshape(H, 1)
    b2dv = np.full((128, 1), b2d, np.float32)

    if "nc" not in _CACHE:
        _CACHE["nc"] = _build()
    nc = _CACHE["nc"]

    bpc = B // NCORES
    in_maps = []
    for c in range(NCORES):
        in_maps.append({
            "xt16": np.ascontiguousarray(x16t[:, c * R:(c + 1) * R]),
            "gu": gumbel_u[c * bpc:(c + 1) * bpc].reshape(R, 2),
            "fu": fix_u[c * bpc:(c + 1) * bpc].reshape(R),
            "w1h": w1h, "emb": emb, "b1c": b1c, "b2dv": b2dv,
        })
    res = bass_utils.run_bass_kernel_spmd(
        nc, in_maps, core_ids=list(range(NCORES)), trace=_trace)
    _CACHE["last_result"] = res

    dec = np.concatenate(
        [res.results[c]["dec"].reshape(bpc, K) for c in range(NCORES)], axis=0)
    keep = np.concatenate(
        [res.results[c]["keep"].reshape(bpc, K) for c in range(NCORES)], axis=0)
    return dec, keep
